# revision 25
# baseline (speedup 1.0000x reference)
"""Trainium2 Bass kernel for nn_AttnLayer (additive attention over history).

Math (per batch b, S = T*N = 8192 positions, A = H = 128):
    c[b]      = cur_h[b] @ Wx_w.T + Wx_b + Wh_b                   (host, tiny)
    pj[a,s]   = alpha * (sum_h Wh[a,h] hist[s,h] + c[b,a])        (PE, [a,s] layout!)
    tnh[a,s]  = tanh(pj/alpha)            ACT share: native tanh (bias+scale free)
                                          DVE share: custom fused op = clamped
                                          odd deg-5 poly p(z)=z(K0+q(K1+q)), q=z^2,
                                          z = clamp(alpha*x, +-Bz)  (|err|<=1.7e-2)
    score[s]  = sum_a v[a] tnh[a,s]       (PE matvec: tnh chunk stationary, v moving,
                                           out free size 1 -> ~free; emitted in
                                           64-wide halves so score lands [64,2] packed)
    esc       = exp(score)  (fp8)         (ACT, accum_out -> per-partition sums)
    attn_h[h] = (sum_s esc[s] hist[s,h]) / sum_s esc[s]           (PE DoubleRow; host divide)
    out[b]    = cur_h[b] + attn_h                                 (host, tiny)

Layouts (host pre-packed, all history fp8):
    histT8[b][p][j*8192+s]       = hist[b, s, 64j+p]     pass-1 moving (DoubleRow k=(p,j))
    histN8[b][p][(i*2+j)*128+h]  = hist[b, 128i+64j+p, h] pass-2 moving (DoubleRow)
The DoubleRow perf mode (both operands fp8, contraction packed 64 partitions x 2)
runs the PE at 0.5 cyc/output-col, and the tiny-weight stationaries make PE cheap;
the kernel is DMA-bound (2 fp8 copies of history ~ 8.4 MB/core) with the tanh
columns split across ACT and DVE to fit inside the DMA window.

Sharding: data-parallel over batch B=32 across 8 cores (4 batches/core).
"""

import os
import sys
from contextlib import ExitStack

import numpy as np
import ml_dtypes

for _p in (
    "/root/.axon_site",
    "/root/.axon_site/_ro/trn_rl_repo",
    "/root/.axon_site/_ro/pypackages",
    "/opt/trn_rl_repo",
):
    if os.path.isdir(_p) and _p not in sys.path:
        sys.path.append(_p)

import concourse.bass as bass  # noqa: E402
import concourse.tile as tile  # noqa: E402
from concourse import bacc, mybir  # noqa: E402
import concourse.bass_utils as bass_utils  # noqa: E402
import concourse.dve_ops as dve_ops  # noqa: E402
from concourse.dve_spec import (  # noqa: E402
    Spec, Src0, Src1, C0, C1, C2, maxx, minn, lower, _has_src1,
)
from concourse.dve_uop import DveOpSpec  # noqa: E402
from concourse.dve_table_gen import dve_ver_for  # noqa: E402

BF16 = mybir.dt.bfloat16
FP8 = mybir.dt.float8e4
F32 = mybir.dt.float32
NPBF16 = ml_dtypes.bfloat16
NPFP8 = ml_dtypes.float8_e4m3

B, T, N, HID, ATTN = 32, 64, 128, 128, 128
NCORES = 8
BL = B // NCORES          # batches per core
S = T * N                 # history positions per batch (8192)
P = 128
HP = 64                   # half partitions (DoubleRow contraction = 64 x 2)
KC = 512                  # chunk columns (1 psum bank; tanh instruction size)
NKC = S // KC             # chunks per batch (16)
NPC = 2                   # histT DMA pieces per batch
PJB = int(os.environ.get("K_PJB", "6"))   # pj psum buffers (banks)
# engine plan per batch: per chunk, 'A' = ACT tanh, 'D' = DVE poly tanh
PLAN = os.environ.get("K_PLAN", "AD" * 8)
LAG = int(os.environ.get("K_LAG", "3"))

# clamped odd deg-5 tanh fit (z = ALPHA*x clamped to +-BZ):
# tanh(x) ~= z*(TK0 + q*(TK1 + q)), q = z*z;  max abs err 1.61e-2
ALPHA = 0.447118
TK0 = 2.107214
TK1 = -2.107472
BZ = 0.983659

_cache = {}


def _register_tanh5():
    """Register the fused clamp+poly tanh DVE op (7 ALU stages, 1 uop)."""
    name = "TANH5_CLAMP_ANT"
    for op in dve_ops.OPS:
        if op.name == name:
            return op
    z = minn(maxx(Src0, C0), C1)
    q = z * z
    body = ((q + C2) * q + Src1) * z

    def ref(in0, in1, c0, c1, c2):
        zz = np.minimum(np.maximum(in0.astype(np.float32), c0), c1)
        qq = zz * zz
        return ((qq + c2) * qq + in1) * zz

    spec = Spec(body=body, reference=ref)
    ver = dve_ver_for("TRN2")
    free = [r for r in range(1, 32) if r not in dve_ops._SUB_OPCODE_FOR_NAME.values()]
    row = free[0]
    s = DveOpSpec(name=name, opcode=row, uops=lower(spec, ver=ver),
                  rd1_en=_has_src1(spec))
    op = dve_ops.DveOp(name, spec, subdim=False, uops_sha={ver: s.sha(ver)})
    dve_ops.OPS.append(op)
    dve_ops._SUB_OPCODE_FOR_NAME[name] = row
    dve_ops.CUSTOM_DVE_SPECS[name] = spec
    return op


TANH5 = _register_tanh5()


def _build_kernel(tc, histT8, histN8, wpack8, v16, acc_out, z_out):
    nc = tc.nc
    AF = mybir.ActivationFunctionType
    DR = mybir.MatmulPerfMode.DoubleRow
    with ExitStack() as ctx:
        wpool = ctx.enter_context(tc.tile_pool(name="w", bufs=1))
        bigT = ctx.enter_context(tc.tile_pool(name="bigT", bufs=BL * NPC + 2))
        bigN = ctx.enter_context(tc.tile_pool(name="bigN", bufs=BL))
        pjp = ctx.enter_context(tc.tile_pool(name="pj", bufs=PJB, space="PSUM"))
        sap = ctx.enter_context(tc.tile_pool(name="sa", bufs=2, space="PSUM"))
        tnhp = ctx.enter_context(tc.tile_pool(name="tnh", bufs=4))
        escp = ctx.enter_context(tc.tile_pool(name="esc", bufs=2))
        zp = ctx.enter_context(tc.tile_pool(name="z", bufs=2))
        accsb = ctx.enter_context(tc.tile_pool(name="accsb", bufs=2))

        # --- small weights first (ACT ring, one combined DMA: doesn't
        # block SP load issue, barely delays first tanh) ---
        # per-batch stationary [65, 2, 128]: rows 0-63 = whT8 (alpha*Wh),
        # row 64 = (j0: fp8-coarse alpha*c[b], j1: residual) bias pair; the
        # moving histT8 carries a matching all-ones row 64 -> bias lands in
        # pj via the same DoubleRow matmul, costing zero extra PE cycles.
        w8 = wpool.tile([HP + 1, BL * 2 * P], FP8, tag="w8")
        nc.scalar.dma_start(
            w8[:].rearrange("p (b m) -> p b m", b=BL),
            wpack8.rearrange("b p m -> p b m"),
        )
        wbs = [
            w8[:, 2 * P * b : 2 * P * (b + 1)].rearrange("p (two m) -> p two m", two=2)
            for b in range(BL)
        ]
        vsb = wpool.tile([P, 1], BF16, tag="v16")
        nc.scalar.dma_start(vsb[:], v16)
        k0t = wpool.tile([P, KC], F32, tag="k0")
        nc.gpsimd.memset(k0t[:], TK0)

        # --- history loads (sync ring), interleaved so histN8[b] lands just
        # before batch b's tail needs it ---
        Tbs = {}
        Nbs = {}

        def load_T(b, npc):
            t = bigT.tile([HP + 1, 2 * (S // npc)], FP8, tag="histT")
            Tbs.setdefault(b, [])
            q = len(Tbs[b])
            src = histT8[b].rearrange("p (two s) -> p two s", two=2)
            nc.sync.dma_start(
                t[:].rearrange("p (two s) -> p two s", two=2),
                src[:, :, (S // npc) * q : (S // npc) * (q + 1)],
            )
            Tbs[b].append((t, S // npc))

        def load_N(b):
            t = bigN.tile([HP, 2 * S], FP8, tag="histN")
            nc.sync.dma_start(t[:], histN8[b])
            Nbs[b] = t

        load_T(0, 4)
        load_T(0, 4)
        load_T(0, 4)
        load_T(0, 4)
        load_T(1, NPC)
        load_T(1, NPC)
        load_N(0)
        load_T(2, NPC)
        load_T(2, NPC)
        load_N(1)
        load_T(3, NPC)
        load_T(3, NPC)
        load_N(2)
        load_N(3)

        def histT_slice(b, s0, ncols):
            """[64, 2, ncols] moving slice for s-range [s0, s0+ncols)."""
            for t, piece_s in Tbs[b]:
                if s0 < piece_s:
                    ap = t[:].rearrange("p (two s) -> p two s", two=2)
                    return ap[:, :, s0 : s0 + ncols]
                s0 -= piece_s
            raise AssertionError("bad slice")

        scoreaccs = {}
        tnhs = {}

        def prod(b, kc):
            """pass-1 chunk: one DoubleRow matmul (bias in row 64) + tanh."""
            eng = PLAN[kc]
            pj = pjp.tile([P, KC], F32, tag="pj")
            nc.tensor.matmul(
                pj[:],
                wbs[b],
                histT_slice(b, KC * kc, KC),
                start=True, stop=True,
                perf_mode=DR,
            )
            tnh = tnhp.tile([P, KC], BF16, tag="tnh")
            if eng == "D":
                nc.vector._custom_dve(
                    TANH5, out=tnh[:], in0=pj[:], in1=k0t[:],
                    s0=-BZ, s1=BZ, imm2=TK1,
                )
            else:
                nc.scalar.activation(
                    tnh[:], pj[:], AF.Tanh, scale=1.0 / ALPHA,
                )
            tnhs[(b, kc)] = tnh

        def matvecs(b, kc):
            """score halves for kilochunk kc: 16 matvecs, out [64,1] each."""
            if kc == 0:
                scoreaccs[b] = sap.tile([HP, 2 * P], F32, tag="sa", name=f"sa{b}")
            sa = scoreaccs[b]
            tnh = tnhs.pop((b, kc))
            for m in range(KC // HP):
                g = (KC // HP) * kc + m          # global half-tile, s in [64g, 64g+64)
                col = (g % 2) * (S // P) + g // 2  # j-major: [2, 64] esc layout
                nc.tensor.matmul(
                    sa[:, col : col + 1],
                    tnh[:, HP * m : HP * (m + 1)],
                    vsb[:],
                    start=True, stop=True,
                )

        escs = {}
        NT = S // P         # pass-2 s-tiles per batch (64)
        P2G = int(os.environ.get("K_P2G", "8"))   # pass-2 emission groups/batch
        P2SPREAD = int(os.environ.get("K_P2SPREAD", "1"))

        def exp_task(b, half):
            """exp of score tiles [32*half, 32*(half+1)) -> fp8 esc + z accum."""
            sa = scoreaccs[b]
            if half == 0:
                esc = escp.tile([HP, P], FP8, tag="esc", name=f"esc{b}")
                zrow = zp.tile([HP, 2], F32, tag="zrow", name=f"zrow{b}")
                escs[b] = (esc, zrow)
            esc, zrow = escs[b]
            hs = NT // 4
            sc_r = sa[:, 0:P].rearrange("p (two i) -> p two i", two=2)
            esc_r = esc[:].rearrange("p (two i) -> p two i", two=2)
            nc.scalar.activation(
                esc_r[:, :, hs * half : hs * (half + 1)],
                sc_r[:, :, hs * half : hs * (half + 1)],
                AF.Exp, accum_out=zrow[:, half : half + 1],
            )
            if half == 1:
                nc.sync.dma_start(z_out[b], zrow[:])

        def p2_task(b, g):
            """pass-2 DoubleRow accumulation, group g of P2G."""
            sa = scoreaccs[b]
            esc_r = escs[b][0][:].rearrange("p (two i) -> p two i", two=2)
            nb_r = Nbs[b][:].rearrange("p (i two h) -> p i two h", two=2, h=P)
            acc = sa[0:1, P : 2 * P]
            gn = NT // P2G
            for i in range(gn * g, gn * (g + 1)):
                nc.tensor.matmul(
                    acc,
                    esc_r[:, :, i : i + 1],
                    nb_r[:, i],
                    start=(i == 0), stop=(i == NT - 1),
                    perf_mode=DR,
                )
            if g == P2G - 1:
                ob = accsb.tile([1, P], F32, tag="ob")
                nc.vector.tensor_copy(ob[:], acc)
                nc.sync.dma_start(acc_out[b : b + 1, :], ob[:])
                scoreaccs.pop(b)
                escs.pop(b)

        # --- software pipeline: producers in order; consumers pop when
        # ready (lagged), earliest-ready first so nothing head-blocks ---
        import heapq

        pend = []
        prod_idx = 0
        seq = 0

        def push(ready, task):
            nonlocal seq
            heapq.heappush(pend, (ready, seq, task))
            seq += 1

        def emit(t):
            if t[0] == "mv":
                matvecs(t[1], t[2])
            elif t[0] == "exp":
                exp_task(t[1], t[2])
            else:
                p2_task(t[1], t[2])

        for b in range(BL):
            for kc in range(NKC):
                prod(b, kc)
                prod_idx += 1
                push(prod_idx + LAG, ("mv", b, kc))
                if kc == NKC // 2 - 1 or kc == NKC - 1:
                    half = kc // (NKC // 2)
                    push(prod_idx + LAG, ("exp", b, half))
                    for gg in range(P2G // 2):
                        g = half * (P2G // 2) + gg
                        push(prod_idx + LAG + 1 + gg * P2SPREAD, ("p2", b, g))
                while pend and pend[0][0] <= prod_idx:
                    emit(heapq.heappop(pend)[2])
        while pend:
            emit(heapq.heappop(pend)[2])


def build():
    if "nc" in _cache:
        return _cache["nc"]
    nc = bacc.Bacc(
        "TRN2",
        target_bir_lowering=False,
        debug=False,
        enable_asserts=True,
        num_devices=NCORES,
    )
    histT8 = nc.dram_tensor("histT8", [BL, HP + 1, 2 * S], FP8, kind="ExternalInput").ap()
    histN8 = nc.dram_tensor("histN8", [BL, HP, 2 * S], FP8, kind="ExternalInput").ap()
    wpack8 = nc.dram_tensor("wpack8", [BL, HP + 1, 2 * P], FP8, kind="ExternalInput").ap()
    v16 = nc.dram_tensor("v16", [P, 1], BF16, kind="ExternalInput").ap()
    acc_out = nc.dram_tensor("acc_out", [BL, P], F32, kind="ExternalOutput").ap()
    z_out = nc.dram_tensor("z_out", [BL, HP, 2], F32, kind="ExternalOutput").ap()

    with tile.TileContext(nc) as tc:
        _build_kernel(tc, histT8, histN8, wpack8, v16, acc_out, z_out)
    nc.compile()
    _cache["nc"] = nc
    return nc


def make_in_maps(cur_h, history_h, Wx_w, Wx_b, Wh_w, Wh_b, v_w):
    """Host-side prep: shard over batch, pre-pack fp8 layouts, fold tiny ops."""
    cur_h = np.asarray(cur_h, np.float32)
    hist = np.asarray(history_h, np.float32).reshape(B, S, HID)
    c = (cur_h @ np.asarray(Wx_w, np.float32).T
         + np.asarray(Wx_b, np.float32)
         + np.asarray(Wh_b, np.float32))                      # [B, A]

    # pass-1 moving: histT8[b, p, j*S + s] = hist[b, s, 64j+p]; row 64 = ones
    hT = np.ascontiguousarray(hist.transpose(0, 2, 1))        # [B, H, S]
    histT8 = np.ones((B, HP + 1, 2 * S), NPFP8)
    histT8[:, :HP] = (hT.reshape(B, 2, HP, S).transpose(0, 2, 1, 3)
                      .reshape(B, HP, 2 * S).astype(NPFP8))
    # pass-2 moving: histN8[b, p, (i*2+j)*128 + h] = hist[b, 128i+64j+p, h]
    histN8 = (hist.reshape(B, T, 2, HP, HID).transpose(0, 3, 1, 2, 4)
              .reshape(B, HP, 2 * S).astype(NPFP8))

    whT = np.asarray(Wh_w, np.float32).T * ALPHA              # [h, a] scaled
    whT8 = (whT.reshape(2, HP, ATTN).transpose(1, 0, 2)
            .reshape(HP, 2 * ATTN)).astype(NPFP8)             # [64, 2*128]

    v16 = np.ascontiguousarray(np.asarray(v_w, np.float32)[:, None]).astype(NPBF16)

    in_maps = []
    for q in range(NCORES):
        bsl = slice(BL * q, BL * (q + 1))
        cq = c[bsl] * ALPHA                                   # [BL, A]
        wpack8 = np.zeros((BL, HP + 1, 2 * ATTN), NPFP8)
        wpack8[:, :HP] = whT8[None]
        coarse = cq.astype(NPFP8)                             # bias row, j=0
        resid = (cq - coarse.astype(np.float32)).astype(NPFP8)  # j=1
        wpack8[:, HP, :ATTN] = coarse
        wpack8[:, HP, ATTN:] = resid
        in_maps.append(
            {
                "histT8": np.ascontiguousarray(histT8[bsl]),
                "histN8": np.ascontiguousarray(histN8[bsl]),
                "wpack8": wpack8,
                "v16": v16,
            }
        )
    return in_maps, cur_h


def finish_host(results, cur):
    outs = []
    for q in range(NCORES):
        acc = results[q]["acc_out"]                           # [BL, P] unnormalized
        z = results[q]["z_out"].reshape(BL, -1).sum(axis=1)   # [BL]
        outs.append(acc / z[:, None])
    attn = np.concatenate(outs, axis=0)
    return (cur + attn).astype(np.float32)


def kernel(cur_h, history_h, Wx_w, Wx_b, Wh_w, Wh_b, v_w):
    nc = build()
    in_maps, cur = make_in_maps(cur_h, history_h, Wx_w, Wx_b, Wh_w, Wh_b, v_w)
    res = bass_utils.run_bass_kernel_spmd(nc, in_maps, core_ids=list(range(NCORES)))
    return finish_host(res.results, cur)


if __name__ == "__main__":
    build()
    print("build ok")
